# revision 1
# baseline (speedup 1.0000x reference)
"""Trainium2 Bass kernel for nn_DiagonalTraining (anti-diagonal per-diag Linear).

out[b, r, c] = sum_{r'} W[d, r - r0(d), r' - r0(d)] * x[b, r', d - r'] + bias,
with d = r + c, over the valid range of r' for diagonal d.

Strategy: shard the 511 independent diagonals across 8 cores (expert-style).
The host packs each core's work into uniform-shape matmul jobs:
  - short diagonals (n <= 128): pair-packed into bins of K=128 (block-diag W),
    one matmul [K=128] x [N=128] per bin, 17 bins/core.
  - long diagonals (128 < n <= 256): one job each, PSUM-accumulated over 2
    K-chunks of 128, N=256 outputs, 32 jobs/core.
Stationary operand = gathered diagonal data xd^T [K, batch=128]; moving
operand = per-diagonal weights [K, N]. PSUM out = [batch=128, N].
Host scatters the packed outputs back to the grid and adds bias.
"""

import sys

sys.path.insert(0, "/opt/trn_rl_repo")

import numpy as np

B, S = 128, 256
D = 2 * S - 1  # 511
NCORES = 8
NSB = 17  # short-diagonal bins per core
NLJ = 32  # long-diagonal jobs per core

USE_BF16 = False  # flipped after precision/perf measurement
USE_F32R = True  # float32r: same fp32 bits, full-rate PE streaming at N>=256
TRACE = False  # test.py sets True to pull exec_time_ns from the NTFF profile
last_results = None


def _geom(d):
    r0 = max(0, d - S + 1)
    n = d + 1 if d < S else 2 * S - 1 - d
    return r0, n


def _job_tables():
    """Static per-core packing tables (indices + masks + scatter targets)."""
    # ---- short bins: 129 real bins + 7 dummies = 136 = 8 * 17
    sbins = []
    for kk in range(1, 64):
        sbins.append([kk - 1, 127 - kk])
        sbins.append([511 - kk, 383 + kk])
    sbins.append([63, 447])
    sbins.append([127])
    sbins.append([383])
    sbins += [[] for _ in range(136 - len(sbins))]
    # ---- long jobs: d in [128, 382] (255) + 1 dummy = 256 = 8 * 32
    ljobs = [[d] for d in range(128, 383)] + [[]]

    cores = []
    for c in range(NCORES):
        my_s = sbins[c::NCORES]
        my_l = ljobs[c::NCORES]
        xds_i = np.zeros((NSB, 128), np.int64)
        xds_m = np.zeros((NSB, 128), np.float32)
        ws_i = np.zeros((NSB, 128, 128), np.int64)
        ws_m = np.zeros((NSB, 128, 128), np.float32)
        tgt_s = np.full((NSB, 128), -1, np.int64)
        for j, bin_ds in enumerate(my_s):
            off = 0
            for d in bin_ds:
                r0, n = _geom(d)
                i = np.arange(n)
                r = r0 + i
                col = d - r
                xds_i[j, off : off + n] = r * S + col
                xds_m[j, off : off + n] = 1.0
                # W[d, m, k] at [k, m] (k = contraction pos, m = output pos)
                ws_i[j, off : off + n, off : off + n] = (
                    d * S * S + i[None, :] * S + i[:, None]
                )
                ws_m[j, off : off + n, off : off + n] = 1.0
                tgt_s[j, off : off + n] = r * S + col
                off += n

        xdl_i = np.zeros((NLJ, 2, 128), np.int64)
        xdl_m = np.zeros((NLJ, 2, 128), np.float32)
        wl_i = np.zeros((NLJ, 2, 128, 256), np.int64)
        wl_m = np.zeros((NLJ, 2, 128, 256), np.float32)
        tgt_l = np.full((NLJ, 256), -1, np.int64)
        for j, job in enumerate(my_l):
            if not job:
                continue
            (d,) = job
            r0, n = _geom(d)
            m = np.arange(256)
            for ch in range(2):
                i = ch * 128 + np.arange(128)
                v = i < n
                r = r0 + np.minimum(i, n - 1)
                xdl_i[j, ch] = (r * S + (d - r)) * v
                xdl_m[j, ch] = v.astype(np.float32)
                mv = (m < n)[None, :] & v[:, None]
                wl_i[j, ch] = (d * S * S + np.minimum(m, n - 1)[None, :] * S + np.minimum(i, n - 1)[:, None]) * mv
                wl_m[j, ch] = mv.astype(np.float32)
            mr = r0 + m[: n]
            tgt_l[j, :n] = mr * S + (d - mr)
        cores.append(
            dict(
                xds_i=xds_i, xds_m=xds_m, ws_i=ws_i, ws_m=ws_m, tgt_s=tgt_s,
                xdl_i=xdl_i, xdl_m=xdl_m, wl_i=wl_i, wl_m=wl_m, tgt_l=tgt_l,
            )
        )
    # bias gather: out_flat[p] += b[d, r - r0(d)] for p = r*S + c, d = r + c
    rr, cc = np.divmod(np.arange(S * S), S)
    dd = rr + cc
    r0v = np.maximum(0, dd - S + 1)
    bidx = dd * S + (rr - r0v)
    return cores, bidx


_TABLES = None
_PROG = {}


def _tables():
    global _TABLES
    if _TABLES is None:
        _TABLES = _job_tables()
    return _TABLES


def _build_program(use_bf16):
    import concourse.bass as bass
    import concourse.mybir as mybir
    import concourse.tile as tile

    f32 = mybir.dt.float32
    if use_bf16:
        dt_in = mybir.dt.bfloat16
    elif USE_F32R:
        dt_in = mybir.dt.float32r
    else:
        dt_in = f32
    nc = bass.Bass()
    bl = nc.dram_tensor("bl", [128, NLJ * 2 * 384], dt_in, kind="ExternalInput")
    bs = nc.dram_tensor("bs", [128, NSB * 256], dt_in, kind="ExternalInput")
    ys = nc.dram_tensor("ys", [128, NSB * 128], f32, kind="ExternalOutput")
    yl = nc.dram_tensor("yl", [128, NLJ * 256], f32, kind="ExternalOutput")

    CH = 4  # L-jobs per load group
    NPS = 6  # psum slots (full banks, cycled)
    SG_BOUNDS = [(0, 8), (8, NSB)]  # S-bin load groups

    # SBUF staging (no reuse -> no WAR deps on input DMAs)
    BTL = [
        nc.alloc_sbuf_tensor(f"btl{g}", [128, CH * 2 * 384], dt_in).ap()
        for g in range(NLJ // CH)
    ]
    BTS = [
        nc.alloc_sbuf_tensor(f"bts{g}", [128, (j1 - j0) * 256], dt_in).ap()
        for g, (j0, j1) in enumerate(SG_BOUNDS)
    ]
    YL = nc.alloc_sbuf_tensor("YL", [128, NLJ * 256], f32).ap()
    YS = nc.alloc_sbuf_tensor("YS", [128, NSB * 128], f32).ap()
    PS = [
        nc.alloc_psum_tensor(f"ps{i}", [128, 512], f32).ap() for i in range(NPS)
    ]

    # unified job list: (required_input_dma_count, n_chunks, lhs/rhs slices, out)
    jobs = []
    for j in range(NLJ):
        g = j // CH
        jj = j % CH
        ops = []
        for ch in range(2):
            o = (jj * 2 + ch) * 384
            ops.append((BTL[g], o))
        jobs.append(("L", g + 1, ops, j))
    n_l_dma = NLJ // CH
    for gi, (j0, j1) in enumerate(SG_BOUNDS):
        for j in range(j0, j1):
            o = (j - j0) * 256
            jobs.append(("S", n_l_dma + gi + 1, [(BTS[gi], o)], j))

    DIN = [
        nc.alloc_semaphore(f"din{i}")
        for i in range(NLJ // CH + len(SG_BOUNDS))
    ]  # one per input DMA (completion order across queues is not FIFO)
    P = nc.alloc_semaphore("P")  # PE job completions
    C = nc.alloc_semaphore("C")  # DVE copy completions
    DO = nc.alloc_semaphore("DO")  # output DMA completions (x16)

    with nc.Block() as block:

        @block.sync
        def _(sync):
            for g in range(n_l_dma):
                sync.dma_start(
                    out=BTL[g][:], in_=bl[:, g * CH * 2 * 384 : (g + 1) * CH * 2 * 384]
                ).then_inc(DIN[g], 16)
            for gi, (j0, j1) in enumerate(SG_BOUNDS):
                sync.dma_start(
                    out=BTS[gi][:], in_=bs[:, j0 * 256 : j1 * 256]
                ).then_inc(DIN[n_l_dma + gi], 16)
            n_out = 0
            for g in range(n_l_dma):
                sync.wait_ge(C, (g + 1) * CH)
                sync.dma_start(
                    out=yl[:, g * CH * 256 : (g + 1) * CH * 256],
                    in_=YL[:, g * CH * 256 : (g + 1) * CH * 256],
                ).then_inc(DO, 16)
                n_out += 1
            for gi, (j0, j1) in enumerate(SG_BOUNDS):
                sync.wait_ge(C, NLJ + j1)
                sync.dma_start(
                    out=ys[:, j0 * 128 : j1 * 128], in_=YS[:, j0 * 128 : j1 * 128]
                ).then_inc(DO, 16)
                n_out += 1
            sync.wait_ge(DO, 16 * n_out)

        @block.tensor
        def _(tensor):
            cur_d = 0
            for ji, (kind, dthr, ops, j) in enumerate(jobs):
                if dthr > cur_d:
                    tensor.wait_ge(DIN[dthr - 1], 16)
                    cur_d = dthr
                if ji >= NPS:
                    tensor.wait_ge(C, ji - NPS + 1)
                ps = PS[ji % NPS]
                if kind == "L":
                    for ch, (bt, o) in enumerate(ops):
                        mm = nc.tensor.matmul(
                            ps[:, 0:256],
                            bt[:, o : o + 128],
                            bt[:, o + 128 : o + 384],
                            start=(ch == 0),
                            stop=(ch == 1),
                        )
                else:
                    (bt, o) = ops[0]
                    mm = nc.tensor.matmul(
                        ps[:, 0:128],
                        bt[:, o : o + 128],
                        bt[:, o + 128 : o + 256],
                        start=True,
                        stop=True,
                    )
                mm.then_inc(P, 1)

        @block.vector
        def _(vector):
            for ji, (kind, dthr, ops, j) in enumerate(jobs):
                vector.wait_ge(P, ji + 1)
                ps = PS[ji % NPS]
                if kind == "L":
                    cp = nc.vector.tensor_copy(
                        YL[:, j * 256 : (j + 1) * 256], ps[:, 0:256]
                    )
                else:
                    cp = nc.vector.tensor_copy(
                        YS[:, j * 128 : (j + 1) * 128], ps[:, 0:128]
                    )
                cp.then_inc(C, 1)

    return nc


def _get_program(use_bf16):
    if use_bf16 not in _PROG:
        _PROG[use_bf16] = _build_program(use_bf16)
    return _PROG[use_bf16]


def _pack_core(t, x_flat, W_flat, np_dt):
    xds = (x_flat[:, t["xds_i"]] * t["xds_m"]).astype(np_dt)  # [B, NSB, 128]
    XDS = xds.transpose(2, 1, 0)  # [128k, NSB, 128b]
    ws = (W_flat[t["ws_i"]] * t["ws_m"]).astype(np_dt)  # [NSB, 128k, 128m]
    WS = ws.transpose(1, 0, 2)  # [128k, NSB, 128m]
    BS = np.concatenate([XDS, WS], axis=2).reshape(128, NSB * 256)
    xdl = (x_flat[:, t["xdl_i"]] * t["xdl_m"]).astype(np_dt)  # [B, NLJ, 2, 128]
    XDL = xdl.transpose(3, 1, 2, 0).reshape(128, NLJ * 2, 128)
    wldat = (W_flat[t["wl_i"]] * t["wl_m"]).astype(np_dt)  # [NLJ, 2, 128, 256]
    WL = wldat.transpose(2, 0, 1, 3).reshape(128, NLJ * 2, 256)
    BL = np.concatenate([XDL, WL], axis=2).reshape(128, NLJ * 2 * 384)
    return {
        "bl": np.ascontiguousarray(BL),
        "bs": np.ascontiguousarray(BS),
    }


def kernel(x, W, b):
    import ml_dtypes
    from concourse.bass_utils import run_bass_kernel_spmd

    x = np.asarray(x, np.float32)
    W = np.asarray(W, np.float32)
    b = np.asarray(b, np.float32)
    cores, bidx = _tables()
    np_dt = ml_dtypes.bfloat16 if USE_BF16 else np.float32
    x_flat = x.reshape(B, S * S)
    W_flat = W.reshape(-1)
    in_maps = [_pack_core(t, x_flat, W_flat, np_dt) for t in cores]
    nc = _get_program(USE_BF16)
    res = run_bass_kernel_spmd(
        nc, in_maps, core_ids=list(range(NCORES)), trace=TRACE
    )
    global last_results
    last_results = res
    out_flat = np.zeros((B, S * S), np.float32)
    for c, t in enumerate(cores):
        ysv = res.results[c]["ys"].reshape(B, -1)
        ylv = res.results[c]["yl"].reshape(B, -1)
        fs = t["tgt_s"].reshape(-1)
        vs = fs >= 0
        out_flat[:, fs[vs]] = ysv[:, vs]
        fl = t["tgt_l"].reshape(-1)
        vl = fl >= 0
        out_flat[:, fl[vl]] = ylv[:, vl]
    out_flat += b.reshape(-1)[bidx][None, :]
    return out_flat.reshape(B, S, S)



# revision 2
# speedup vs baseline: 1.2431x; 1.2431x over previous
"""Trainium2 Bass kernel for nn_DiagonalTraining (anti-diagonal per-diag Linear).

out[b, r, c] = sum_{k} W[d, m, k] * x[b, r0(d)+k, d-r0(d)-k] + bias[d, m],
with d = r + c, m = r - r0(d).

Strategy: shard the 511 independent diagonals across 8 cores. All streams
are bf16 (rel-err budget 2e-2; measured ~2.4e-3), which halves HBM traffic
vs f32 and runs the PE at 1 cycle/row for any N.

Long diagonals (n > 128, d in [128,382], 255 of them) are grouped into
complementary PAIRS with nA + nB = 384 so the two chunk-1 k-ranges
(aA = nA-128, aB = nB-128, aA+aB = 128) exactly fill one 128-partition
stationary tile.  Per pair, 3 stationary xd tiles [128k x 128b] and 4 W
moving blocks:
  psum[:, 0:NA]    = xd0A.T @ W0A + xdp.T @ W1A   (chunk0 + chunk1 of A)
  psum[:, NA:NA+NB]= xd0B.T @ W0B + xdp.T @ W1B
W0A/W0B are sent at (near-)exact width; W1A/W1B carry structural zero rows
(the other pair member's partitions).

SPMD runs ONE program on 8 cores, so per-core column layouts must agree:
the 127 pairs + the standalone n=256 diagonal are sorted by size into 16
"slots"; slot u has one pair per core and a uniform (NA_u, NB_u) padded to
the slot max (pad <= 4 cols since sorted).

Short diagonals (n <= 128) keep the pair-packed block-diagonal bins of the
f32 baseline: 129 real bins + 7 dummies = 8 x 17, each one [128k x 128m]
matmul.

Outputs are written bf16, exact-packed, and unpacked/scattered on host.
Input DMAs issue from the Activation HWDGE ring, output DMAs from the SP
ring so the two streams interleave at the SDMA packet level.
"""

import sys

sys.path.insert(0, "/opt/trn_rl_repo")

import numpy as np

B, S = 128, 256
D = 2 * S - 1  # 511
NCORES = 8
NSLOT = 16  # long pair-slots per core
NSB = 17  # short bins per core
NPS = 8  # psum banks cycled over jobs

TRACE = False  # test.py sets True to pull exec_time_ns from the NTFF profile
last_results = None

# job schedule: slots 0..14, then 17 short bins, then slot 15 (small tail)
LGROUPS = [(0, 6), (6, 12), (12, 15), (15, 16)]  # slot ranges per long DMA
N_JOBS = 15 + NSB + 1  # 33


def _geom(d):
    r0 = max(0, d - S + 1)
    n = d + 1 if d < S else 2 * S - 1 - d
    return r0, n


def _layout():
    """Global slot structure: slots[u][c] = (dA, dB|None), uniform shapes."""
    pairs = [(d, 382 - d) for d in range(128, 191)]  # left: nA+nB = 384
    pairs += [(d, 638 - d) for d in range(320, 383)]  # right (A = smaller n)
    pairs.append((191, 319))  # the two n=192 diagonals
    pairs.sort(key=lambda p: -_geom(p[0])[1])  # by nA desc
    slots = [pairs[7 + 8 * u : 15 + 8 * u] for u in range(15)]
    # slot 15: 7 biggest pairs on cores 0-6, the full n=256 diag on core 7
    slots.append(pairs[:7] + [(255, None)])

    shapes = []
    for ent in slots:
        NA = max(_geom(dA)[1] for dA, _ in ent)
        NB = max(_geom(dB)[1] if dB is not None else 0 for _, dB in ent)
        shapes.append((NA, NB))

    col0, CL = [], 0
    for NA, NB in shapes:
        col0.append(CL)
        CL += 384 + 2 * (NA + NB)
    ocol0, OL = [], 0
    for NA, NB in shapes:
        ocol0.append(OL)
        OL += NA + NB
    return slots, shapes, col0, CL, ocol0, OL


_SLOTS, _SHAPES, _COL0, CL, _OCOL0, OL = _layout()


def _short_bins():
    sbins = []
    for kk in range(1, 64):
        sbins.append([kk - 1, 127 - kk])
        sbins.append([511 - kk, 383 + kk])
    sbins.append([63, 447])
    sbins.append([127])
    sbins.append([383])
    sbins += [[] for _ in range(136 - len(sbins))]
    return sbins


def _wblk(d_, n_, koff, plo, phi, width):
    """W moving block [128, width]: [p, m] = W[d_, m, koff + p - plo]
    valid for p in [plo, phi) and m < n_; zero elsewhere."""
    p = np.arange(128)[:, None]
    m = np.arange(width)[None, :]
    kk = koff + (p - plo)
    msk = (p >= plo) & (p < phi) & (m < n_)
    idx = d_ * S * S + m * S + np.clip(kk, 0, S - 1)
    return np.where(msk, idx, 0).astype(np.int64), msk


def _diag_flat(d, kvals):
    """Flat x/grid index of diagonal d at positions kvals."""
    r0, n = _geom(d)
    r = r0 + kvals
    return r * S + (d - r)


def _core_tables():
    """Static per-core packing tables."""
    cores = []
    for c in range(NCORES):
        xdb = []  # (dstcol, idx[128], valid)
        wb = []  # (dstcol, idx[128, w], msk[128, w])
        tgt_l = np.full(OL, -1, np.int64)
        k = np.arange(128)
        for u in range(NSLOT):
            dA, dB = _SLOTS[u][c]
            NA, NB = _SHAPES[u]
            c0 = _COL0[u]
            r0A, nA = _geom(dA)
            aA = nA - 128
            xdb.append((c0, _diag_flat(dA, k), True))
            if dB is not None:
                r0B, nB = _geom(dB)
                xdb.append((c0 + 128, _diag_flat(dB, k), True))
            else:
                nB = 0
                xdb.append((c0 + 128, np.zeros(128, np.int64), False))
            # mixed chunk-1 stationary: p < aA -> A k=128+p, else B k=128+(p-aA)
            iA = _diag_flat(dA, np.minimum(128 + k, nA - 1))
            if dB is not None:
                iB = _diag_flat(dB, np.clip(128 + (k - aA), 0, nB - 1))
            else:
                iB = np.zeros(128, np.int64)
            xdb.append((c0 + 256, np.where(k < aA, iA, iB), True))
            # W moving blocks
            i0, m0 = _wblk(dA, nA, 0, 0, 128, NA)
            wb.append((c0 + 384, i0, m0))
            i1, m1 = _wblk(dB, nB, 0, 0, 128, NB) if dB is not None else (
                np.zeros((128, NB), np.int64), np.zeros((128, NB), bool))
            wb.append((c0 + 384 + NA, i1, m1))
            i2, m2 = _wblk(dA, nA, 128, 0, aA, NA)
            wb.append((c0 + 384 + NA + NB, i2, m2))
            i3, m3 = _wblk(dB, nB, 128, aA, 128, NB) if dB is not None else (
                np.zeros((128, NB), np.int64), np.zeros((128, NB), bool))
            wb.append((c0 + 384 + 2 * NA + NB, i3, m3))
            # output scatter targets
            tgt_l[_OCOL0[u] : _OCOL0[u] + nA] = _diag_flat(dA, np.arange(nA))
            if dB is not None:
                tgt_l[_OCOL0[u] + NA : _OCOL0[u] + NA + nB] = _diag_flat(
                    dB, np.arange(nB))

        # ---- short bins (same packing as the f32 baseline) ----
        sbins = _short_bins()
        my_s = sbins[c::NCORES]
        xds_i = np.zeros((NSB, 128), np.int64)
        xds_m = np.zeros((NSB, 128), np.float32)
        ws_i = np.zeros((NSB, 128, 128), np.int64)
        ws_m = np.zeros((NSB, 128, 128), np.float32)
        tgt_s = np.full((NSB, 128), -1, np.int64)
        for j, bin_ds in enumerate(my_s):
            off = 0
            for d in bin_ds:
                r0, n = _geom(d)
                i = np.arange(n)
                r = r0 + i
                col = d - r
                xds_i[j, off : off + n] = r * S + col
                xds_m[j, off : off + n] = 1.0
                ws_i[j, off : off + n, off : off + n] = (
                    d * S * S + i[None, :] * S + i[:, None]
                )
                ws_m[j, off : off + n, off : off + n] = 1.0
                tgt_s[j, off : off + n] = r * S + col
                off += n
        cores.append(
            dict(xdb=xdb, wb=wb, tgt_l=tgt_l, xds_i=xds_i, xds_m=xds_m,
                 ws_i=ws_i, ws_m=ws_m, tgt_s=tgt_s)
        )
    rr, cc = np.divmod(np.arange(S * S), S)
    dd = rr + cc
    r0v = np.maximum(0, dd - S + 1)
    bidx = dd * S + (rr - r0v)
    return cores, bidx


_TABLES = None
_PROG = None


def _tables():
    global _TABLES
    if _TABLES is None:
        _TABLES = _core_tables()
    return _TABLES


def _jobs():
    """Unified job order: (kind, index, input-DMA threshold)."""
    jobs = []
    for g, (u0, u1) in enumerate(LGROUPS[:3]):
        for u in range(u0, u1):
            jobs.append(("L", u, g + 1))
    for j in range(NSB):
        jobs.append(("S", j, 4))
    jobs.append(("L", 15, 5))
    return jobs


def _build_program():
    import concourse.bass as bass
    import concourse.mybir as mybir

    f32 = mybir.dt.float32
    bf16 = mybir.dt.bfloat16
    nc = bass.Bass()
    dl = nc.dram_tensor("dl", [128, CL], bf16, kind="ExternalInput")
    ds = nc.dram_tensor("ds", [128, NSB * 256], bf16, kind="ExternalInput")
    yl = nc.dram_tensor("yl", [128, OL], bf16, kind="ExternalOutput")
    ys = nc.dram_tensor("ys", [128, NSB * 128], bf16, kind="ExternalOutput")

    # staging (one tensor per input DMA -> no WAR deps)
    BTL = [
        nc.alloc_sbuf_tensor(
            f"btl{g}", [128, _COL0[u1 - 1] + 384 + 2 * sum(_SHAPES[u1 - 1]) - _COL0[u0]], bf16
        ).ap()
        for g, (u0, u1) in enumerate(LGROUPS)
    ]
    BTS = nc.alloc_sbuf_tensor("bts", [128, NSB * 256], bf16).ap()
    YL = nc.alloc_sbuf_tensor("YL", [128, OL], bf16).ap()
    YS = nc.alloc_sbuf_tensor("YS", [128, NSB * 128], bf16).ap()
    PS = [nc.alloc_psum_tensor(f"ps{i}", [128, 512], f32).ap() for i in range(NPS)]

    DIN = [nc.alloc_semaphore(f"din{i}") for i in range(5)]
    P = nc.alloc_semaphore("P")
    C = nc.alloc_semaphore("C")
    DO = nc.alloc_semaphore("DO")

    jobs = _jobs()
    # copies completed after job index ji, per group boundary
    out_events = [
        (6, "yl", 0, _OCOL0[6]),
        (12, "yl", _OCOL0[6], _OCOL0[12]),
        (15, "yl", _OCOL0[12], _OCOL0[15]),
        (15 + NSB, "ys", 0, NSB * 128),
        (N_JOBS, "yl", _OCOL0[15], OL),
    ]

    with nc.Block() as block:

        @block.scalar
        def _(scalar):
            # input DMAs on the ACT HWDGE ring
            for g, (u0, u1) in enumerate(LGROUPS[:3]):
                scalar.dma_start(
                    out=BTL[g][:], in_=dl[:, _COL0[u0] : _COL0[u1] if u1 < NSLOT else CL]
                ).then_inc(DIN[g], 16)
            scalar.dma_start(out=BTS[:], in_=ds[:, :]).then_inc(DIN[3], 16)
            scalar.dma_start(out=BTL[3][:], in_=dl[:, _COL0[15] : CL]).then_inc(
                DIN[4], 16
            )

        @block.sync
        def _(sync):
            # output DMAs on the SP HWDGE ring
            for thr, which, o0, o1 in out_events:
                sync.wait_ge(C, thr)
                if which == "yl":
                    sync.dma_start(out=yl[:, o0:o1], in_=YL[:, o0:o1]).then_inc(DO, 16)
                else:
                    sync.dma_start(out=ys[:, o0:o1], in_=YS[:, o0:o1]).then_inc(DO, 16)
            sync.wait_ge(DO, 16 * len(out_events))

        @block.tensor
        def _(tensor):
            cur_d = 0
            for ji, (kind, idx, dthr) in enumerate(jobs):
                if dthr > cur_d:
                    tensor.wait_ge(DIN[dthr - 1], 16)
                    cur_d = dthr
                if ji >= NPS:
                    tensor.wait_ge(C, ji - NPS + 1)
                ps = PS[ji % NPS]
                if kind == "L":
                    u = idx
                    g = next(g for g, (u0, u1) in enumerate(LGROUPS) if u0 <= u < u1)
                    o = _COL0[u] - _COL0[LGROUPS[g][0]]
                    NA, NB = _SHAPES[u]
                    bt = BTL[g]
                    xa = bt[:, o : o + 128]
                    xb = bt[:, o + 128 : o + 256]
                    xp = bt[:, o + 256 : o + 384]
                    wA0 = bt[:, o + 384 : o + 384 + NA]
                    wB0 = bt[:, o + 384 + NA : o + 384 + NA + NB]
                    wA1 = bt[:, o + 384 + NA + NB : o + 384 + 2 * NA + NB]
                    wB1 = bt[:, o + 384 + 2 * NA + NB : o + 384 + 2 * NA + 2 * NB]
                    nc.tensor.matmul(ps[:, 0:NA], xa, wA0, start=True, stop=False)
                    nc.tensor.matmul(ps[:, 0:NA], xp, wA1, start=False, stop=True)
                    nc.tensor.matmul(
                        ps[:, NA : NA + NB], xb, wB0, start=True, stop=False
                    )
                    mm = nc.tensor.matmul(
                        ps[:, NA : NA + NB], xp, wB1, start=False, stop=True
                    )
                else:
                    o = idx * 256
                    mm = nc.tensor.matmul(
                        ps[:, 0:128],
                        BTS[:, o : o + 128],
                        BTS[:, o + 128 : o + 256],
                        start=True,
                        stop=True,
                    )
                mm.then_inc(P, 1)

        @block.vector
        def _(vector):
            for ji, (kind, idx, dthr) in enumerate(jobs):
                vector.wait_ge(P, ji + 1)
                ps = PS[ji % NPS]
                if kind == "L":
                    NA, NB = _SHAPES[idx]
                    o = _OCOL0[idx]
                    cp = nc.vector.tensor_copy(
                        YL[:, o : o + NA + NB], ps[:, 0 : NA + NB]
                    )
                else:
                    cp = nc.vector.tensor_copy(
                        YS[:, idx * 128 : (idx + 1) * 128], ps[:, 0:128]
                    )
                cp.then_inc(C, 1)

    return nc


def _get_program():
    global _PROG
    if _PROG is None:
        _PROG = _build_program()
    return _PROG


def _pack_core(t, x_flat, W_flat, np_bf16):
    dl = np.zeros((128, CL), np.float32)
    for c0, idx, valid in t["xdb"]:
        if valid:
            dl[:, c0 : c0 + 128] = x_flat[:, idx].T
    for c0, idx, msk in t["wb"]:
        w = idx.shape[1]
        if w:
            dl[:, c0 : c0 + w] = W_flat[idx] * msk
    xds = x_flat[:, t["xds_i"]] * t["xds_m"]  # [B, NSB, 128]
    ws = W_flat[t["ws_i"]] * t["ws_m"]  # [NSB, 128k, 128m]
    dsb = np.zeros((128, NSB * 256), np.float32)
    dsb3 = dsb.reshape(128, NSB, 256)
    dsb3[:, :, 0:128] = xds.transpose(2, 1, 0)
    dsb3[:, :, 128:256] = ws.transpose(1, 0, 2)
    return {"dl": dl.astype(np_bf16), "ds": dsb.astype(np_bf16)}


def kernel(x, W, b):
    import ml_dtypes
    from concourse.bass_utils import run_bass_kernel_spmd

    x = np.asarray(x, np.float32)
    W = np.asarray(W, np.float32)
    b = np.asarray(b, np.float32)
    cores, bidx = _tables()
    x_flat = x.reshape(B, S * S)
    W_flat = W.reshape(-1)
    np_bf16 = ml_dtypes.bfloat16
    in_maps = [_pack_core(t, x_flat, W_flat, np_bf16) for t in cores]
    nc = _get_program()
    res = run_bass_kernel_spmd(nc, in_maps, core_ids=list(range(NCORES)), trace=TRACE)
    global last_results
    last_results = res
    out_flat = np.zeros((B, S * S), np.float32)
    for c, t in enumerate(cores):
        ylv = np.asarray(res.results[c]["yl"], np.float32).reshape(B, -1)
        fl = t["tgt_l"]
        vl = fl >= 0
        out_flat[:, fl[vl]] = ylv[:, vl]
        ysv = np.asarray(res.results[c]["ys"], np.float32).reshape(B, -1)
        fs = t["tgt_s"].reshape(-1)
        vs = fs >= 0
        out_flat[:, fs[vs]] = ysv[:, vs]
    out_flat += b.reshape(-1)[bidx][None, :]
    return out_flat.reshape(B, S, S)


# revision 5
# speedup vs baseline: 1.4124x; 1.1363x over previous
"""Trainium2 Bass kernel for nn_DiagonalTraining (anti-diagonal per-diag Linear).

out[b, r, c] = sum_{k} W[d, m, k] * x[b, r0(d)+k, d-r0(d)-k] + bias[d, m],
with d = r + c, m = r - r0(d).

Strategy: shard the 511 independent diagonals across 8 cores. All streams
are bf16 (rel-err budget 2e-2; measured ~2.4e-3), which halves HBM traffic
vs f32 and runs the PE at 1 cycle/row for any N.

Long diagonals (n > 128, d in [128,382], 255 of them) are grouped into
complementary PAIRS with nA + nB = 384 so the two chunk-1 k-ranges
(aA = nA-128, aB = nB-128, aA+aB = 128) exactly fill one 128-partition
stationary tile.  Per pair, 3 stationary xd tiles [128k x 128b] and 4 W
moving blocks:
  psum[:, 0:NA]    = xd0A.T @ W0A + xdp.T @ W1A   (chunk0 + chunk1 of A)
  psum[:, NA:NA+NB]= xd0B.T @ W0B + xdp.T @ W1B
W0A/W0B are sent at (near-)exact width; W1A/W1B carry structural zero rows
(the other pair member's partitions).

SPMD runs ONE program on 8 cores, so per-core column layouts must agree:
the 127 pairs + the standalone n=256 diagonal are sorted by size into 16
"slots"; slot u has one pair per core and a uniform (NA_u, NB_u) padded to
the slot max (pad <= 4 cols since sorted).

Short diagonals (n <= 128) keep the pair-packed block-diagonal bins of the
f32 baseline: 129 real bins + 7 dummies = 8 x 17, each one [128k x 128m]
matmul.

Outputs are written bf16, exact-packed, and unpacked/scattered on host.
Input DMAs issue from the Activation HWDGE ring, output DMAs from the SP
ring so the two streams interleave at the SDMA packet level.
"""

import sys

sys.path.insert(0, "/opt/trn_rl_repo")

import numpy as np

B, S = 128, 256
D = 2 * S - 1  # 511
NCORES = 8
NSLOT = 16  # long pair-slots per core
NSB = 17  # short bins per core
NPS = 8  # psum banks cycled over jobs

TRACE = False  # test.py sets True to pull exec_time_ns from the NTFF profile
last_results = None

# job schedule: slots 0..14, then 17 short bins, then slot 15 (small tail)
# long-slot DMA groups alternate between the two HWDGE rings (scalar=ACT,
# sync=SP) so both queue rows stream concurrently (~400 GB/s vs ~282 single)
LGROUPS = [(0, 3), (3, 6), (6, 9), (9, 12), (12, 15), (15, 16)]
LG_RING = ["scalar", "sync", "scalar", "sync", "scalar", "scalar"]
N_JOBS = 15 + NSB + 1  # 33


def _geom(d):
    r0 = max(0, d - S + 1)
    n = d + 1 if d < S else 2 * S - 1 - d
    return r0, n


def _layout():
    """Global slot structure: slots[u][c] = (dA, dB|None), uniform shapes."""
    pairs = [(d, 382 - d) for d in range(128, 191)]  # left: nA+nB = 384
    pairs += [(d, 638 - d) for d in range(320, 383)]  # right (A = smaller n)
    pairs.append((191, 319))  # the two n=192 diagonals
    pairs.sort(key=lambda p: -_geom(p[0])[1])  # by nA desc
    slots = [pairs[7 + 8 * u : 15 + 8 * u] for u in range(15)]
    # slot 15: 7 biggest pairs on cores 0-6, the full n=256 diag on core 7
    slots.append(pairs[:7] + [(255, None)])

    shapes = []
    for ent in slots:
        NA = max(_geom(dA)[1] for dA, _ in ent)
        NB = max(_geom(dB)[1] if dB is not None else 0 for _, dB in ent)
        shapes.append((NA, NB))

    col0, CL = [], 0
    for NA, NB in shapes:
        col0.append(CL)
        CL += 384 + 2 * (NA + NB)
    ocol0, OL = [], 0
    for NA, NB in shapes:
        ocol0.append(OL)
        OL += NA + NB
    return slots, shapes, col0, CL, ocol0, OL


_SLOTS, _SHAPES, _COL0, CL, _OCOL0, OL = _layout()


def _short_bins():
    sbins = []
    for kk in range(1, 64):
        sbins.append([kk - 1, 127 - kk])
        sbins.append([511 - kk, 383 + kk])
    sbins.append([63, 447])
    sbins.append([127])
    sbins.append([383])
    sbins += [[] for _ in range(136 - len(sbins))]
    return sbins


def _wblk(d_, n_, koff, plo, phi, width):
    """W moving block [128, width]: [p, m] = W[d_, m, koff + p - plo]
    valid for p in [plo, phi) and m < n_; zero elsewhere."""
    p = np.arange(128)[:, None]
    m = np.arange(width)[None, :]
    kk = koff + (p - plo)
    msk = (p >= plo) & (p < phi) & (m < n_)
    idx = d_ * S * S + m * S + np.clip(kk, 0, S - 1)
    return np.where(msk, idx, 0).astype(np.int64), msk


def _diag_flat(d, kvals):
    """Flat x/grid index of diagonal d at positions kvals."""
    r0, n = _geom(d)
    r = r0 + kvals
    return r * S + (d - r)


def _core_tables():
    """Static per-core packing tables."""
    cores = []
    for c in range(NCORES):
        xdb = []  # (dstcol, idx[128], valid)
        wb = []  # (dstcol, idx[128, w], msk[128, w])
        tgt_l = np.full(OL, -1, np.int64)
        k = np.arange(128)
        for u in range(NSLOT):
            dA, dB = _SLOTS[u][c]
            NA, NB = _SHAPES[u]
            c0 = _COL0[u]
            r0A, nA = _geom(dA)
            aA = nA - 128
            xdb.append((c0, _diag_flat(dA, k), True))
            if dB is not None:
                r0B, nB = _geom(dB)
                xdb.append((c0 + 128, _diag_flat(dB, k), True))
            else:
                nB = 0
                xdb.append((c0 + 128, np.zeros(128, np.int64), False))
            # mixed chunk-1 stationary: p < aA -> A k=128+p, else B k=128+(p-aA)
            iA = _diag_flat(dA, np.minimum(128 + k, nA - 1))
            if dB is not None:
                iB = _diag_flat(dB, np.clip(128 + (k - aA), 0, nB - 1))
            else:
                iB = np.zeros(128, np.int64)
            xdb.append((c0 + 256, np.where(k < aA, iA, iB), True))
            # W moving blocks
            i0, m0 = _wblk(dA, nA, 0, 0, 128, NA)
            wb.append((c0 + 384, i0, m0))
            i1, m1 = _wblk(dB, nB, 0, 0, 128, NB) if dB is not None else (
                np.zeros((128, NB), np.int64), np.zeros((128, NB), bool))
            wb.append((c0 + 384 + NA, i1, m1))
            i2, m2 = _wblk(dA, nA, 128, 0, aA, NA)
            wb.append((c0 + 384 + NA + NB, i2, m2))
            i3, m3 = _wblk(dB, nB, 128, aA, 128, NB) if dB is not None else (
                np.zeros((128, NB), np.int64), np.zeros((128, NB), bool))
            wb.append((c0 + 384 + 2 * NA + NB, i3, m3))
            # output scatter targets
            tgt_l[_OCOL0[u] : _OCOL0[u] + nA] = _diag_flat(dA, np.arange(nA))
            if dB is not None:
                tgt_l[_OCOL0[u] + NA : _OCOL0[u] + NA + nB] = _diag_flat(
                    dB, np.arange(nB))

        # ---- short bins (same packing as the f32 baseline) ----
        sbins = _short_bins()
        my_s = sbins[c::NCORES]
        xds_i = np.zeros((NSB, 128), np.int64)
        xds_m = np.zeros((NSB, 128), np.float32)
        ws_i = np.zeros((NSB, 128, 128), np.int64)
        ws_m = np.zeros((NSB, 128, 128), np.float32)
        tgt_s = np.full((NSB, 128), -1, np.int64)
        for j, bin_ds in enumerate(my_s):
            off = 0
            for d in bin_ds:
                r0, n = _geom(d)
                i = np.arange(n)
                r = r0 + i
                col = d - r
                xds_i[j, off : off + n] = r * S + col
                xds_m[j, off : off + n] = 1.0
                ws_i[j, off : off + n, off : off + n] = (
                    d * S * S + i[None, :] * S + i[:, None]
                )
                ws_m[j, off : off + n, off : off + n] = 1.0
                tgt_s[j, off : off + n] = r * S + col
                off += n
        cores.append(
            dict(xdb=xdb, wb=wb, tgt_l=tgt_l, xds_i=xds_i, xds_m=xds_m,
                 ws_i=ws_i, ws_m=ws_m, tgt_s=tgt_s)
        )
    rr, cc = np.divmod(np.arange(S * S), S)
    dd = rr + cc
    r0v = np.maximum(0, dd - S + 1)
    bidx = dd * S + (rr - r0v)
    return cores, bidx


_TABLES = None
_PROG = None


def _tables():
    global _TABLES
    if _TABLES is None:
        _TABLES = _core_tables()
    return _TABLES


def _jobs():
    """Unified job order: (kind, index, input-DMA threshold).

    DIN order: 0..4 = long groups (0,3),(3,6),(6,9),(9,12),(12,15);
    5 = shorts; 6 = slot 15."""
    jobs = []
    for g, (u0, u1) in enumerate(LGROUPS[:5]):
        for u in range(u0, u1):
            jobs.append(("L", u, g + 1))
    for j in range(NSB):
        jobs.append(("S", j, 6))
    jobs.append(("L", 15, 7))
    return jobs


def _build_program():
    import concourse.bass as bass
    import concourse.mybir as mybir

    f32 = mybir.dt.float32
    bf16 = mybir.dt.bfloat16
    nc = bass.Bass()
    dl = nc.dram_tensor("dl", [128, CL], bf16, kind="ExternalInput")
    ds = nc.dram_tensor("ds", [128, NSB * 256], bf16, kind="ExternalInput")
    yl = nc.dram_tensor("yl", [128, OL], bf16, kind="ExternalOutput")
    ys = nc.dram_tensor("ys", [128, NSB * 128], bf16, kind="ExternalOutput")

    # staging (one tensor per input DMA -> no WAR deps)
    BTL = [
        nc.alloc_sbuf_tensor(
            f"btl{g}", [128, _COL0[u1 - 1] + 384 + 2 * sum(_SHAPES[u1 - 1]) - _COL0[u0]], bf16
        ).ap()
        for g, (u0, u1) in enumerate(LGROUPS)
    ]
    BTS = nc.alloc_sbuf_tensor("bts", [128, NSB * 256], bf16).ap()
    YL = nc.alloc_sbuf_tensor("YL", [128, OL], bf16).ap()
    YS = nc.alloc_sbuf_tensor("YS", [128, NSB * 128], bf16).ap()
    PS = [nc.alloc_psum_tensor(f"ps{i}", [128, 512], f32).ap() for i in range(NPS)]

    DIN = [nc.alloc_semaphore(f"din{i}") for i in range(7)]
    P = nc.alloc_semaphore("P")
    C = nc.alloc_semaphore("C")
    DO = nc.alloc_semaphore("DO")

    jobs = _jobs()
    # copies completed after job index ji, per group boundary
    out_events = [
        (3, "yl", 0, _OCOL0[3]),
        (6, "yl", _OCOL0[3], _OCOL0[6]),
        (12, "yl", _OCOL0[6], _OCOL0[12]),
        (15, "yl", _OCOL0[12], _OCOL0[15]),
        (15 + NSB, "ys", 0, NSB * 128),
        (N_JOBS, "yl", _OCOL0[15], OL),
    ]

    def _lg_dma(eng, g):
        u0, u1 = LGROUPS[g]
        eng.dma_start(
            out=BTL[g][:], in_=dl[:, _COL0[u0] : _COL0[u1] if u1 < NSLOT else CL]
        ).then_inc(DIN[g if g < 5 else 6], 16)

    with nc.Block(no_gpsimd_drain=True) as block:

        @block.scalar
        def _(scalar):
            # input DMAs, ACT HWDGE ring: groups 0, 2, 4, slot15
            for g in (0, 2, 4, 5):
                _lg_dma(scalar, g)

        @block.sync
        def _(sync):
            # input DMAs, SP HWDGE ring: groups 1, 3, shorts
            for g in (1, 3):
                _lg_dma(sync, g)
            sync.dma_start(out=BTS[:], in_=ds[:, :]).then_inc(DIN[5], 16)

        @block.gpsimd
        def _(gpsimd):
            # output DMAs on the SWDGE ring (3rd concurrent queue row)
            for thr, which, o0, o1 in out_events:
                gpsimd.wait_ge(C, thr)
                if which == "yl":
                    gpsimd.dma_start(out=yl[:, o0:o1], in_=YL[:, o0:o1]).then_inc(
                        DO, 16
                    )
                else:
                    gpsimd.dma_start(out=ys[:, o0:o1], in_=YS[:, o0:o1]).then_inc(
                        DO, 16
                    )
            gpsimd.wait_ge(DO, 16 * len(out_events))

        @block.tensor
        def _(tensor):
            cur_d = 0
            for ji, (kind, idx, dthr) in enumerate(jobs):
                if dthr > cur_d:
                    tensor.wait_ge(DIN[dthr - 1], 16)
                    cur_d = dthr
                if ji >= NPS:
                    tensor.wait_ge(C, ji - NPS + 1)
                ps = PS[ji % NPS]
                if kind == "L":
                    u = idx
                    g = next(g for g, (u0, u1) in enumerate(LGROUPS) if u0 <= u < u1)
                    o = _COL0[u] - _COL0[LGROUPS[g][0]]
                    NA, NB = _SHAPES[u]
                    bt = BTL[g]
                    xa = bt[:, o : o + 128]
                    xb = bt[:, o + 128 : o + 256]
                    xp = bt[:, o + 256 : o + 384]
                    wA0 = bt[:, o + 384 : o + 384 + NA]
                    wB0 = bt[:, o + 384 + NA : o + 384 + NA + NB]
                    wA1 = bt[:, o + 384 + NA + NB : o + 384 + 2 * NA + NB]
                    wB1 = bt[:, o + 384 + 2 * NA + NB : o + 384 + 2 * NA + 2 * NB]
                    nc.tensor.matmul(ps[:, 0:NA], xa, wA0, start=True, stop=False)
                    nc.tensor.matmul(ps[:, 0:NA], xp, wA1, start=False, stop=True)
                    nc.tensor.matmul(
                        ps[:, NA : NA + NB], xb, wB0, start=True, stop=False
                    )
                    mm = nc.tensor.matmul(
                        ps[:, NA : NA + NB], xp, wB1, start=False, stop=True
                    )
                else:
                    o = idx * 256
                    mm = nc.tensor.matmul(
                        ps[:, 0:128],
                        BTS[:, o : o + 128],
                        BTS[:, o + 128 : o + 256],
                        start=True,
                        stop=True,
                    )
                mm.then_inc(P, 1)

        @block.vector
        def _(vector):
            for ji, (kind, idx, dthr) in enumerate(jobs):
                vector.wait_ge(P, ji + 1)
                ps = PS[ji % NPS]
                if kind == "L":
                    NA, NB = _SHAPES[idx]
                    o = _OCOL0[idx]
                    cp = nc.vector.tensor_copy(
                        YL[:, o : o + NA + NB], ps[:, 0 : NA + NB]
                    )
                else:
                    cp = nc.vector.tensor_copy(
                        YS[:, idx * 128 : (idx + 1) * 128], ps[:, 0:128]
                    )
                cp.then_inc(C, 1)

    return nc


def _get_program():
    global _PROG
    if _PROG is None:
        _PROG = _build_program()
    return _PROG


def _pack_core(t, x_flat, W_flat, np_bf16):
    dl = np.zeros((128, CL), np.float32)
    for c0, idx, valid in t["xdb"]:
        if valid:
            dl[:, c0 : c0 + 128] = x_flat[:, idx].T
    for c0, idx, msk in t["wb"]:
        w = idx.shape[1]
        if w:
            dl[:, c0 : c0 + w] = W_flat[idx] * msk
    xds = x_flat[:, t["xds_i"]] * t["xds_m"]  # [B, NSB, 128]
    ws = W_flat[t["ws_i"]] * t["ws_m"]  # [NSB, 128k, 128m]
    dsb = np.zeros((128, NSB * 256), np.float32)
    dsb3 = dsb.reshape(128, NSB, 256)
    dsb3[:, :, 0:128] = xds.transpose(2, 1, 0)
    dsb3[:, :, 128:256] = ws.transpose(1, 0, 2)
    return {"dl": dl.astype(np_bf16), "ds": dsb.astype(np_bf16)}


def kernel(x, W, b):
    import ml_dtypes
    from concourse.bass_utils import run_bass_kernel_spmd

    x = np.asarray(x, np.float32)
    W = np.asarray(W, np.float32)
    b = np.asarray(b, np.float32)
    cores, bidx = _tables()
    x_flat = x.reshape(B, S * S)
    W_flat = W.reshape(-1)
    np_bf16 = ml_dtypes.bfloat16
    in_maps = [_pack_core(t, x_flat, W_flat, np_bf16) for t in cores]
    nc = _get_program()
    res = run_bass_kernel_spmd(nc, in_maps, core_ids=list(range(NCORES)), trace=TRACE)
    global last_results
    last_results = res
    out_flat = np.zeros((B, S * S), np.float32)
    for c, t in enumerate(cores):
        ylv = np.asarray(res.results[c]["yl"], np.float32).reshape(B, -1)
        fl = t["tgt_l"]
        vl = fl >= 0
        out_flat[:, fl[vl]] = ylv[:, vl]
        ysv = np.asarray(res.results[c]["ys"], np.float32).reshape(B, -1)
        fs = t["tgt_s"].reshape(-1)
        vs = fs >= 0
        out_flat[:, fs[vs]] = ysv[:, vs]
    out_flat += b.reshape(-1)[bidx][None, :]
    return out_flat.reshape(B, S, S)


# revision 11
# speedup vs baseline: 1.4837x; 1.0505x over previous
"""Trainium2 Bass kernel for nn_DiagonalTraining (anti-diagonal per-diag Linear).

out[b, r, c] = sum_{k} W[d, m, k] * x[b, r0(d)+k, d-r0(d)-k] + bias[d, m],
with d = r + c, m = r - r0(d).

Strategy: shard the 511 independent diagonals across 8 cores. All streams
are bf16 (rel-err budget 2e-2; measured ~2.4e-3), which halves HBM traffic
vs f32 and runs the PE at 1 cycle/row for any N.

Long diagonals (n > 128, d in [128,382], 255 of them) are grouped into
complementary PAIRS with nA + nB = 384 so the two chunk-1 k-ranges
(aA = nA-128, aB = nB-128, aA+aB = 128) exactly fill one 128-partition
stationary tile.  Per pair, 3 stationary xd tiles [128k x 128b] and 4 W
moving blocks:
  psum[:, 0:NA]    = xd0A.T @ W0A + xdp.T @ W1A   (chunk0 + chunk1 of A)
  psum[:, NA:NA+NB]= xd0B.T @ W0B + xdp.T @ W1B
W0A/W0B are sent at (near-)exact width; W1A/W1B carry structural zero rows
(the other pair member's partitions).

SPMD runs ONE program on 8 cores, so per-core column layouts must agree:
the 127 pairs + the standalone n=256 diagonal are sorted by size into 16
"slots"; slot u has one pair per core and a uniform (NA_u, NB_u) padded to
the slot max (pad <= 4 cols since sorted).

Short diagonals (n <= 128) keep the pair-packed block-diagonal bins of the
f32 baseline: 129 real bins + 7 dummies = 8 x 17, each one [128k x 128m]
matmul.

Outputs are written bf16, exact-packed, and unpacked/scattered on host.
Input DMAs issue from the Activation HWDGE ring, output DMAs from the SP
ring so the two streams interleave at the SDMA packet level.
"""

import sys

sys.path.insert(0, "/opt/trn_rl_repo")

import numpy as np

B, S = 128, 256
D = 2 * S - 1  # 511
NCORES = 8
NSLOT = 16  # long pair-slots per core
NSB = 17  # short bins per core
NPS = 8  # psum banks cycled over jobs

TRACE = False  # test.py sets True to pull exec_time_ns from the NTFF profile
last_results = None

# long-slot DMA groups alternate between the two HWDGE rings (scalar=ACT,
# sync=SP) so both queue rows stream concurrently (~400 GB/s vs ~282 single).
# scalar ring: G0, G2, G4, G5; sync ring: G1, shorts, G3.
# job order: slots 0-8, shorts, slots 9-15 (matches per-ring arrival order)
LGROUPS = [(0, 3), (3, 6), (6, 9), (9, 12), (12, 15), (15, 16)]
N_JOBS = 15 + NSB + 1  # 33


def _geom(d):
    r0 = max(0, d - S + 1)
    n = d + 1 if d < S else 2 * S - 1 - d
    return r0, n


def _layout():
    """Global slot structure: slots[u][c] = (dA, dB|None), uniform shapes."""
    pairs = [(d, 382 - d) for d in range(128, 191)]  # left: nA+nB = 384
    pairs += [(d, 638 - d) for d in range(320, 383)]  # right (A = smaller n)
    pairs.append((191, 319))  # the two n=192 diagonals
    pairs.sort(key=lambda p: -_geom(p[0])[1])  # by nA desc
    rslots = [pairs[7 + 8 * u : 15 + 8 * u] for u in range(15)]
    # standalone slot: 7 biggest pairs on cores 0-6, the n=256 diag on core 7
    sx = pairs[:7] + [(255, None)]
    # job order ends with the smallest regular slots; the wide standalone
    # slot sits at index 12 so the tail stays small
    slots = rslots[:12] + [sx] + rslots[12:]

    shapes = []
    for ent in slots:
        NA = max(_geom(dA)[1] for dA, _ in ent)
        NB = max(_geom(dB)[1] if dB is not None else 0 for _, dB in ent)
        shapes.append((NA, NB))

    col0, CL = [], 0
    for NA, NB in shapes:
        col0.append(CL)
        CL += 384 + 2 * (NA + NB)
    ocol0, OL = [], 0
    for NA, NB in shapes:
        ocol0.append(OL)
        OL += NA + NB
    return slots, shapes, col0, CL, ocol0, OL


_SLOTS, _SHAPES, _COL0, CL, _OCOL0, OL = _layout()


def _short_bins():
    sbins = []
    for kk in range(1, 64):
        sbins.append([kk - 1, 127 - kk])
        sbins.append([511 - kk, 383 + kk])
    sbins.append([63, 447])
    sbins.append([127])
    sbins.append([383])
    sbins += [[] for _ in range(136 - len(sbins))]
    return sbins


def _wblk(d_, n_, koff, plo, phi, width):
    """W moving block [128, width]: [p, m] = W[d_, m, koff + p - plo]
    valid for p in [plo, phi) and m < n_; zero elsewhere."""
    p = np.arange(128)[:, None]
    m = np.arange(width)[None, :]
    kk = koff + (p - plo)
    msk = (p >= plo) & (p < phi) & (m < n_)
    idx = d_ * S * S + m * S + np.clip(kk, 0, S - 1)
    return np.where(msk, idx, 0).astype(np.int64), msk


def _diag_flat(d, kvals):
    """Flat x/grid index of diagonal d at positions kvals."""
    r0, n = _geom(d)
    r = r0 + kvals
    return r * S + (d - r)


def _core_tables():
    """Static per-core packing tables."""
    cores = []
    for c in range(NCORES):
        xdb = []  # (dstcol, idx[128], valid)
        wb = []  # (dstcol, idx[128, w], msk[128, w])
        tgt_l = np.full(OL, -1, np.int64)
        k = np.arange(128)
        for u in range(NSLOT):
            dA, dB = _SLOTS[u][c]
            NA, NB = _SHAPES[u]
            c0 = _COL0[u]
            r0A, nA = _geom(dA)
            aA = nA - 128
            xdb.append((c0, _diag_flat(dA, k), True))
            if dB is not None:
                r0B, nB = _geom(dB)
                xdb.append((c0 + 128, _diag_flat(dB, k), True))
            else:
                nB = 0
                xdb.append((c0 + 128, np.zeros(128, np.int64), False))
            # mixed chunk-1 stationary: p < aA -> A k=128+p, else B k=128+(p-aA)
            iA = _diag_flat(dA, np.minimum(128 + k, nA - 1))
            if dB is not None:
                iB = _diag_flat(dB, np.clip(128 + (k - aA), 0, nB - 1))
            else:
                iB = np.zeros(128, np.int64)
            xdb.append((c0 + 256, np.where(k < aA, iA, iB), True))
            # W moving blocks
            i0, m0 = _wblk(dA, nA, 0, 0, 128, NA)
            wb.append((c0 + 384, i0, m0))
            i1, m1 = _wblk(dB, nB, 0, 0, 128, NB) if dB is not None else (
                np.zeros((128, NB), np.int64), np.zeros((128, NB), bool))
            wb.append((c0 + 384 + NA, i1, m1))
            i2, m2 = _wblk(dA, nA, 128, 0, aA, NA)
            wb.append((c0 + 384 + NA + NB, i2, m2))
            i3, m3 = _wblk(dB, nB, 128, aA, 128, NB) if dB is not None else (
                np.zeros((128, NB), np.int64), np.zeros((128, NB), bool))
            wb.append((c0 + 384 + 2 * NA + NB, i3, m3))
            # output scatter targets
            tgt_l[_OCOL0[u] : _OCOL0[u] + nA] = _diag_flat(dA, np.arange(nA))
            if dB is not None:
                tgt_l[_OCOL0[u] + NA : _OCOL0[u] + NA + nB] = _diag_flat(
                    dB, np.arange(nB))

        # ---- short bins (same packing as the f32 baseline) ----
        sbins = _short_bins()
        my_s = sbins[c::NCORES]
        xds_i = np.zeros((NSB, 128), np.int64)
        xds_m = np.zeros((NSB, 128), np.float32)
        ws_i = np.zeros((NSB, 128, 128), np.int64)
        ws_m = np.zeros((NSB, 128, 128), np.float32)
        tgt_s = np.full((NSB, 128), -1, np.int64)
        for j, bin_ds in enumerate(my_s):
            off = 0
            for d in bin_ds:
                r0, n = _geom(d)
                i = np.arange(n)
                r = r0 + i
                col = d - r
                xds_i[j, off : off + n] = r * S + col
                xds_m[j, off : off + n] = 1.0
                ws_i[j, off : off + n, off : off + n] = (
                    d * S * S + i[None, :] * S + i[:, None]
                )
                ws_m[j, off : off + n, off : off + n] = 1.0
                tgt_s[j, off : off + n] = r * S + col
                off += n
        cores.append(
            dict(xdb=xdb, wb=wb, tgt_l=tgt_l, xds_i=xds_i, xds_m=xds_m,
                 ws_i=ws_i, ws_m=ws_m, tgt_s=tgt_s)
        )
    rr, cc = np.divmod(np.arange(S * S), S)
    dd = rr + cc
    r0v = np.maximum(0, dd - S + 1)
    bidx = dd * S + (rr - r0v)
    return cores, bidx


_TABLES = None
_PROG = None


def _tables():
    global _TABLES
    if _TABLES is None:
        _TABLES = _core_tables()
    return _TABLES


def _jobs():
    """Unified job order: (kind, index, input-DMA-sem index+1).

    DIN order: 0=G0, 1=G1, 2=G2, 3=shorts, 4=G3, 5=G4, 6=G5."""
    jobs = []
    for g in range(3):  # slots 0-8 (G0 scalar, G1 sync, G2 scalar)
        for u in range(*LGROUPS[g]):
            jobs.append(("L", u, g + 1))
    for j in range(NSB):  # shorts (sync, after G1)
        jobs.append(("S", j, 4))
    for g in range(3, 6):  # slots 9-15 (G3 sync, G4 scalar, G5 scalar)
        for u in range(*LGROUPS[g]):
            jobs.append(("L", u, g + 2))
    return jobs


def _cnt(k, e):
    """#copies on engine e (0=DVE, 1=ACT) among jobs 0..k (alternating)."""
    return (k + 2 - e) // 2 if k >= 0 else 0


def _build_program():
    import concourse.bass as bass
    import concourse.mybir as mybir

    f32 = mybir.dt.float32
    bf16 = mybir.dt.bfloat16
    nc = bass.Bass()
    dl = nc.dram_tensor("dl", [128, CL], bf16, kind="ExternalInput")
    ds = nc.dram_tensor("ds", [128, NSB * 256], bf16, kind="ExternalInput")
    yl = nc.dram_tensor("yl", [128, OL], bf16, kind="ExternalOutput")
    ys = nc.dram_tensor("ys", [128, NSB * 128], bf16, kind="ExternalOutput")

    # staging (one tensor per input DMA -> no WAR deps)
    BTL = [
        nc.alloc_sbuf_tensor(
            f"btl{g}", [128, _COL0[u1 - 1] + 384 + 2 * sum(_SHAPES[u1 - 1]) - _COL0[u0]], bf16
        ).ap()
        for g, (u0, u1) in enumerate(LGROUPS)
    ]
    BTS = nc.alloc_sbuf_tensor("bts", [128, NSB * 256], bf16).ap()
    YL = nc.alloc_sbuf_tensor("YL", [128, OL], bf16).ap()
    YS = nc.alloc_sbuf_tensor("YS", [128, NSB * 128], bf16).ap()
    PS = [nc.alloc_psum_tensor(f"ps{i}", [128, 512], f32).ap() for i in range(NPS)]

    DIN = [nc.alloc_semaphore(f"din{i}") for i in range(7)]
    P = nc.alloc_semaphore("P")
    CV = nc.alloc_semaphore("CV")  # DVE copy completions (even jobs)
    CA = nc.alloc_semaphore("CA")  # ACT copy completions (odd jobs)
    DO = nc.alloc_semaphore("DO")

    jobs = _jobs()
    # (last-job-index, tensor, col range, ring) — early outs ride the SWDGE
    # ring (HWDGE rings are busy with inputs); the tail outs ride SP HWDGE
    out_events = [
        (2, "yl", 0, _OCOL0[3], "gpsimd"),
        (5, "yl", _OCOL0[3], _OCOL0[6], "gpsimd"),
        (8, "yl", _OCOL0[6], _OCOL0[9], "gpsimd"),
        (8 + NSB, "ys", 0, NSB * 128, "sync"),
        (11 + NSB, "yl", _OCOL0[9], _OCOL0[12], "gpsimd"),
        (14 + NSB, "yl", _OCOL0[12], _OCOL0[15], "sync"),
        (N_JOBS - 1, "yl", _OCOL0[15], OL, "sync"),
    ]

    def _lg_dma(eng, g, din):
        u0, u1 = LGROUPS[g]
        eng.dma_start(
            out=BTL[g][:], in_=dl[:, _COL0[u0] : _COL0[u1] if u1 < NSLOT else CL]
        ).then_inc(DIN[din], 16)

    def _out_dma(eng, ev):
        k, which, o0, o1, _ = ev
        eng.wait_ge(CV, _cnt(k, 0))
        eng.wait_ge(CA, _cnt(k, 1))
        t, st = (yl, YL) if which == "yl" else (ys, YS)
        eng.dma_start(out=t[:, o0:o1], in_=st[:, o0:o1]).then_inc(DO, 16)

    with nc.Block(no_gpsimd_drain=True) as block:

        @block.sync
        def _(sync):
            # input DMAs, SP HWDGE ring: G1, shorts, G3; then tail outputs
            _lg_dma(sync, 1, 1)
            sync.dma_start(out=BTS[:], in_=ds[:, :]).then_inc(DIN[3], 16)
            _lg_dma(sync, 3, 4)
            for ev in out_events:
                if ev[4] == "sync":
                    _out_dma(sync, ev)
            sync.wait_ge(DO, 16 * len(out_events))

        @block.gpsimd
        def _(gpsimd):
            # early output DMAs on the SWDGE ring (3rd concurrent queue row)
            for ev in out_events:
                if ev[4] == "gpsimd":
                    _out_dma(gpsimd, ev)

        @block.scalar
        def _(scalar):
            # input DMAs, ACT HWDGE ring: G0, G2, G4, G5; then odd-job copies
            _lg_dma(scalar, 0, 0)
            _lg_dma(scalar, 2, 2)
            _lg_dma(scalar, 4, 5)
            _lg_dma(scalar, 5, 6)
            for ji, (kind, idx, dthr) in enumerate(jobs):
                if ji % 2 != 1:
                    continue
                scalar.wait_ge(P, ji + 1)
                ps = PS[ji % NPS]
                if kind == "L":
                    NA, NB = _SHAPES[idx]
                    o = _OCOL0[idx]
                    cp = scalar.copy(YL[:, o : o + NA + NB], ps[:, 0 : NA + NB])
                else:
                    cp = scalar.copy(
                        YS[:, idx * 128 : (idx + 1) * 128], ps[:, 0:128]
                    )
                cp.then_inc(CA, 1)

        @block.tensor
        def _(tensor):
            cur_d = 0
            for ji, (kind, idx, dthr) in enumerate(jobs):
                if dthr > cur_d:
                    tensor.wait_ge(DIN[dthr - 1], 16)
                    cur_d = dthr
                if ji >= NPS:
                    prev = ji - NPS
                    tensor.wait_ge(CV if prev % 2 == 0 else CA, _cnt(prev, prev % 2))
                ps = PS[ji % NPS]
                if kind == "L":
                    u = idx
                    g = next(g for g, (u0, u1) in enumerate(LGROUPS) if u0 <= u < u1)
                    o = _COL0[u] - _COL0[LGROUPS[g][0]]
                    NA, NB = _SHAPES[u]
                    bt = BTL[g]
                    xa = bt[:, o : o + 128]
                    xb = bt[:, o + 128 : o + 256]
                    xp = bt[:, o + 256 : o + 384]
                    wA0 = bt[:, o + 384 : o + 384 + NA]
                    wB0 = bt[:, o + 384 + NA : o + 384 + NA + NB]
                    wA1 = bt[:, o + 384 + NA + NB : o + 384 + 2 * NA + NB]
                    wB1 = bt[:, o + 384 + 2 * NA + NB : o + 384 + 2 * NA + 2 * NB]
                    nc.tensor.matmul(ps[:, 0:NA], xa, wA0, start=True, stop=False)
                    nc.tensor.matmul(ps[:, 0:NA], xp, wA1, start=False, stop=True)
                    nc.tensor.matmul(
                        ps[:, NA : NA + NB], xb, wB0, start=True, stop=False
                    )
                    mm = nc.tensor.matmul(
                        ps[:, NA : NA + NB], xp, wB1, start=False, stop=True
                    )
                else:
                    o = idx * 256
                    mm = nc.tensor.matmul(
                        ps[:, 0:128],
                        BTS[:, o : o + 128],
                        BTS[:, o + 128 : o + 256],
                        start=True,
                        stop=True,
                    )
                mm.then_inc(P, 1)

        @block.vector
        def _(vector):
            for ji, (kind, idx, dthr) in enumerate(jobs):
                if ji % 2 != 0:
                    continue
                vector.wait_ge(P, ji + 1)
                ps = PS[ji % NPS]
                if kind == "L":
                    NA, NB = _SHAPES[idx]
                    o = _OCOL0[idx]
                    cp = nc.vector.tensor_copy(
                        YL[:, o : o + NA + NB], ps[:, 0 : NA + NB]
                    )
                else:
                    cp = nc.vector.tensor_copy(
                        YS[:, idx * 128 : (idx + 1) * 128], ps[:, 0:128]
                    )
                cp.then_inc(CV, 1)

    return nc


def _get_program():
    global _PROG
    if _PROG is None:
        _PROG = _build_program()
    return _PROG


def _pack_core(t, x_flat, W_flat, np_bf16):
    dl = np.zeros((128, CL), np.float32)
    for c0, idx, valid in t["xdb"]:
        if valid:
            dl[:, c0 : c0 + 128] = x_flat[:, idx].T
    for c0, idx, msk in t["wb"]:
        w = idx.shape[1]
        if w:
            dl[:, c0 : c0 + w] = W_flat[idx] * msk
    xds = x_flat[:, t["xds_i"]] * t["xds_m"]  # [B, NSB, 128]
    ws = W_flat[t["ws_i"]] * t["ws_m"]  # [NSB, 128k, 128m]
    dsb = np.zeros((128, NSB * 256), np.float32)
    dsb3 = dsb.reshape(128, NSB, 256)
    dsb3[:, :, 0:128] = xds.transpose(2, 1, 0)
    dsb3[:, :, 128:256] = ws.transpose(1, 0, 2)
    return {"dl": dl.astype(np_bf16), "ds": dsb.astype(np_bf16)}


def kernel(x, W, b):
    import ml_dtypes
    from concourse.bass_utils import run_bass_kernel_spmd

    x = np.asarray(x, np.float32)
    W = np.asarray(W, np.float32)
    b = np.asarray(b, np.float32)
    cores, bidx = _tables()
    x_flat = x.reshape(B, S * S)
    W_flat = W.reshape(-1)
    np_bf16 = ml_dtypes.bfloat16
    in_maps = [_pack_core(t, x_flat, W_flat, np_bf16) for t in cores]
    nc = _get_program()
    res = run_bass_kernel_spmd(nc, in_maps, core_ids=list(range(NCORES)), trace=TRACE)
    global last_results
    last_results = res
    out_flat = np.zeros((B, S * S), np.float32)
    for c, t in enumerate(cores):
        ylv = np.asarray(res.results[c]["yl"], np.float32).reshape(B, -1)
        fl = t["tgt_l"]
        vl = fl >= 0
        out_flat[:, fl[vl]] = ylv[:, vl]
        ysv = np.asarray(res.results[c]["ys"], np.float32).reshape(B, -1)
        fs = t["tgt_s"].reshape(-1)
        vs = fs >= 0
        out_flat[:, fs[vs]] = ysv[:, vs]
    out_flat += b.reshape(-1)[bidx][None, :]
    return out_flat.reshape(B, S, S)
